# revision 4
# baseline (speedup 1.0000x reference)
"""MAGAT GNN message-passing kernel for 8 Trainium2 NeuronCores.

Math: the reference applies Sinkhorn-Knopp to adj0 but only ever uses the
result via `adj > 0` — and Sinkhorn preserves the zero/positive pattern
exactly in fp32 (0/s == 0, pos/pos can't underflow at these magnitudes).
So the device kernel skips Sinkhorn and uses (adj0 > 0) as the softmax
mask. Softmax is computed without max-subtraction (e is bounded by ~±4,
exp can't overflow) and the row-sum is fused into the attention matmul as
an extra ones-column of the weight matrix. The attention matmul runs in
bf16: the residual x0 (O(1)) dominates h_prime (O(0.01)), so bf16
rounding perturbs the final output by only ~3e-5 relative.

Sharding: 8 cores = 4 heads x 2 row-halves. Each core gets its head's
adjacency slice pre-transposed on host to [j=4096, i=2048] so the softmax
reduction over j lands on the PE contraction (partition) axis. x0 is
rolled per-core so "own rows" are always rows 0..2048 — keeps the SPMD
program identical across cores.

Engine balance per core: DMA 32MB (~90us); ACT: one Exp pass over all
elements plus Prelu on 2/3 of chunks; DVE: mask (is_gt*mult) everywhere
plus rank-1 max-of-exps (exp(leaky(x)) == max(exp(x), exp(.2x)), and
exp(.2(s_i+d_j)) factors as exp(.2 s_i)*exp(.2 d_j)) on 1/3 of chunks.
"""

import numpy as np
from contextlib import ExitStack

import concourse.bacc as bacc
import concourse.mybir as mybir
import concourse.tile as tile
import concourse.masks as masks
from concourse.bass_utils import run_bass_kernel_spmd

F32 = mybir.dt.float32
BF16 = mybir.dt.bfloat16
N, F, H, D = 4096, 128, 4, 128
NH = N // 2          # own rows per core
NC = N // 128        # 32 j-chunks
NM = NH // 128       # 16 output row-tiles
ALPHA = 0.2

# chunks where the leaky-relu runs on DVE (rank-1 trick) instead of ACT
DVE_CHUNK = lambda jc: (jc % 3 == 2)

_cache = {}


def _build():
    nc = bacc.Bacc("TRN2", target_bir_lowering=False, debug=False)
    adjT = nc.dram_tensor("adjT", [N, NH], F32, kind="ExternalInput").ap()
    x0r = nc.dram_tensor("x0r", [N, F], F32, kind="ExternalInput").ap()
    w = nc.dram_tensor("w", [F, D], F32, kind="ExternalInput").ap()
    asrc = nc.dram_tensor("asrc", [D, 1], F32, kind="ExternalInput").ap()
    adst = nc.dram_tensor("adst", [D, 1], F32, kind="ExternalInput").ap()
    out = nc.dram_tensor("out", [NH, D], F32, kind="ExternalOutput").ap()

    with tile.TileContext(nc) as tc, ExitStack() as ctx:
        const = ctx.enter_context(tc.tile_pool(name="const", bufs=1))

        # persistent tiles
        x0_sb = const.tile([128, NC * F], F32)        # x0 rows chunked [p, c, f]
        x03 = x0_sb[:].rearrange("p (c f) -> p c f", c=NC)
        whp = const.tile([128, NC * (D + 1)], BF16)   # [Wh | 1] per j-chunk, bf16
        whp3 = whp[:].rearrange("p (c q) -> p c q", c=NC)
        esb = const.tile([128, NH], F32)              # e_src bcast along partitions
        ed_sb = const.tile([128, NC], F32)            # e_dst, col per j-chunk
        a2 = const.tile([128, NH], F32)               # exp(0.2*e_src) bcast
        b2 = const.tile([128, NC], F32)               # exp(0.2*e_dst)

        with ExitStack() as sctx:
            setup = sctx.enter_context(tc.tile_pool(name="setup", bufs=2))
            spsum = sctx.enter_context(tc.tile_pool(name="spsum", bufs=2, space="PSUM"))

            ident = setup.tile([128, 128], F32)
            masks.make_identity(nc, ident[:])
            w_sb = setup.tile([F, D], F32)
            nc.sync.dma_start(w_sb[:], w)
            asrc_sb = setup.tile([D, 1], F32)
            nc.sync.dma_start(asrc_sb[:], asrc)
            adst_sb = setup.tile([D, 1], F32)
            nc.sync.dma_start(adst_sb[:], adst)

            nc.sync.dma_start(
                x03[:, :, :], x0r.rearrange("(c p) f -> p c f", p=128))

            # x0T[f, n] via PE transpose per 128-chunk
            x0T = setup.tile([128, N], F32)
            for c in range(NC):
                pst = spsum.tile([128, 128], F32, tag="sps", name="pst")
                nc.tensor.transpose(pst[:], x03[:, c, :], ident[:])
                nc.scalar.copy(x0T[:, c * 128:(c + 1) * 128], pst[:])

            # Wh chunks -> whp cols 0..128 (cast to bf16); ones col at 128
            for c in range(NC):
                psw = spsum.tile([128, D], F32, tag="sps", name="psw")
                nc.tensor.matmul(psw[:], lhsT=x0T[:, c * 128:(c + 1) * 128],
                                 rhs=w_sb[:], start=True, stop=True)
                nc.vector.tensor_copy(whp3[:, c, 0:D], psw[:])
            nc.vector.memset(whp3[:, :, D], 1.0)

            # WhT[d, n]
            whT = setup.tile([128, N], F32)
            for g in range(N // 512):
                psq = spsum.tile([128, 512], F32, tag="sps", name="psq")
                nc.tensor.matmul(psq[:], lhsT=w_sb[:],
                                 rhs=x0T[:, g * 512:(g + 1) * 512],
                                 start=True, stop=True)
                nc.scalar.copy(whT[:, g * 512:(g + 1) * 512], psq[:])

            # e_src (own rows only) as a [1, NH] row
            es_row = setup.tile([1, NH], F32)
            for g in range(NH // 512):
                pse = spsum.tile([1, 512], F32, tag="sps", name="pse")
                nc.tensor.matmul(pse[:], lhsT=asrc_sb[:],
                                 rhs=whT[:, g * 512:(g + 1) * 512],
                                 start=True, stop=True)
                nc.vector.tensor_copy(es_row[:, g * 512:(g + 1) * 512], pse[:])

            # e_dst per j-chunk -> ed_sb[:, c]
            for c in range(NC):
                psd = spsum.tile([128, 1], F32, tag="sps", name="psd")
                nc.tensor.matmul(psd[:], lhsT=whT[:, c * 128:(c + 1) * 128],
                                 rhs=adst_sb[:], start=True, stop=True)
                nc.vector.tensor_copy(ed_sb[:, c:c + 1], psd[:])

            # esb = broadcast es_row across 128 partitions (ones ⊗ es_row)
            ones_row = setup.tile([1, 128], F32)
            nc.vector.memset(ones_row[:], 1.0)
            for g in range(NH // 512):
                psb = spsum.tile([128, 512], F32, tag="sps", name="psb")
                nc.tensor.matmul(psb[:], lhsT=ones_row[:],
                                 rhs=es_row[:, g * 512:(g + 1) * 512],
                                 start=True, stop=True)
                nc.scalar.copy(esb[:, g * 512:(g + 1) * 512], psb[:])

            # rank-1 factors for the DVE leaky path
            nc.scalar.activation(a2[:], esb[:],
                                 mybir.ActivationFunctionType.Exp, scale=0.2)
            nc.scalar.activation(b2[:], ed_sb[:],
                                 mybir.ActivationFunctionType.Exp, scale=0.2)

        # steady state: one loop over 32 j-chunks, full i-width 2048
        work = ctx.enter_context(tc.tile_pool(name="work", bufs=2))
        atp = ctx.enter_context(tc.tile_pool(name="atp", bufs=4))
        pmp = ctx.enter_context(tc.tile_pool(name="pmp", bufs=3))
        epil = ctx.enter_context(tc.tile_pool(name="epil", bufs=2))
        mpsum = ctx.enter_context(tc.tile_pool(name="mpsum", bufs=1, space="PSUM"))

        # 8 PSUM banks, two [128,129] accumulation groups per bank
        pss = [mpsum.tile([128, 2 * (D + 1)], F32, tag=f"acc{b}", name=f"acc{b}")
               for b in range(8)]

        def acc_view(g):
            return pss[g // 2][:, (g % 2) * (D + 1):(g % 2 + 1) * (D + 1)]

        for jc in range(NC):
            at = atp.tile([128, NH], F32, tag="at")
            nc.sync.dma_start(at[:], adjT[jc * 128:(jc + 1) * 128, :])
            if not DVE_CHUNK(jc):
                el = work.tile([128, NH], F32, tag="el")
                nc.scalar.activation(el[:], esb[:],
                                     mybir.ActivationFunctionType.Prelu,
                                     bias=ed_sb[:, jc:jc + 1], scale=1.0,
                                     alpha=ALPHA)
                p1 = work.tile([128, NH], F32, tag="p1")
                nc.scalar.activation(p1[:], el[:],
                                     mybir.ActivationFunctionType.Exp)
            else:
                p0 = work.tile([128, NH], F32, tag="el")
                nc.scalar.activation(p0[:], esb[:],
                                     mybir.ActivationFunctionType.Exp,
                                     bias=ed_sb[:, jc:jc + 1])
                p1 = work.tile([128, NH], F32, tag="p1")
                # p1 = max(exp(e), exp(.2 e)) with exp(.2 e) = a2 * b2[jc]
                nc.vector.scalar_tensor_tensor(
                    out=p1[:], in0=a2[:], scalar=b2[:, jc:jc + 1], in1=p0[:],
                    op0=mybir.AluOpType.mult, op1=mybir.AluOpType.max)
            pm = pmp.tile([128, NH], BF16, tag="pm")
            nc.vector.scalar_tensor_tensor(
                out=pm[:], in0=at[:], scalar=0.0, in1=p1[:],
                op0=mybir.AluOpType.is_gt, op1=mybir.AluOpType.mult)
            for g in range(NM):
                nc.tensor.matmul(acc_view(g), lhsT=pm[:, g * 128:(g + 1) * 128],
                                 rhs=whp3[:, jc, :],
                                 start=(jc == 0), stop=(jc == NC - 1))

        for g in range(NM):
            ps = acc_view(g)
            rec = epil.tile([128, 1], F32, tag="rec")
            nc.vector.reciprocal(rec[:], ps[:, D:D + 1])
            hp = epil.tile([128, D], F32, tag="hp")
            nc.vector.tensor_scalar_mul(hp[:], ps[:, 0:D], rec[:])
            # elu(x) = max(x, exp(min(x,0)) - 1)
            t1 = epil.tile([128, D], F32, tag="t1")
            nc.vector.tensor_scalar_min(t1[:], hp[:], 0.0)
            ex1 = epil.tile([128, D], F32, tag="ex1")
            nc.scalar.activation(ex1[:], t1[:],
                                 mybir.ActivationFunctionType.Exp)
            el1 = epil.tile([128, D], F32, tag="el1")
            nc.vector.scalar_tensor_tensor(
                out=el1[:], in0=ex1[:], scalar=-1.0, in1=hp[:],
                op0=mybir.AluOpType.add, op1=mybir.AluOpType.max)
            # residual + second elu
            r = epil.tile([128, D], F32, tag="r")
            nc.vector.tensor_add(r[:], el1[:], x03[:, g, :])
            t2 = epil.tile([128, D], F32, tag="t2")
            nc.vector.tensor_scalar_min(t2[:], r[:], 0.0)
            ex2 = epil.tile([128, D], F32, tag="ex2")
            nc.scalar.activation(ex2[:], t2[:],
                                 mybir.ActivationFunctionType.Exp)
            y = epil.tile([128, D], F32, tag="y")
            nc.vector.scalar_tensor_tensor(
                out=y[:], in0=ex2[:], scalar=-1.0, in1=r[:],
                op0=mybir.AluOpType.add, op1=mybir.AluOpType.max)
            nc.sync.dma_start(out[g * 128:(g + 1) * 128, :], y[:])

    nc.compile()
    return nc


def _get_nc():
    if "nc" not in _cache:
        _cache["nc"] = _build()
    return _cache["nc"]


def kernel(x0, adj0, W, a_src, a_dst):
    nc = _get_nc()
    in_maps = []
    for c in range(8):
        h, half = c // 2, c % 2
        i0 = half * NH
        a = adj0[h, i0:i0 + NH, :]
        if i0:
            a = np.concatenate([a[:, i0:], a[:, :i0]], axis=1)
            xr = np.concatenate([x0[i0:], x0[:i0]], axis=0)
        else:
            xr = x0
        in_maps.append(dict(
            adjT=np.ascontiguousarray(a.T),
            x0r=np.ascontiguousarray(xr),
            w=np.ascontiguousarray(W[h]),
            asrc=np.ascontiguousarray(a_src[h][:, None]),
            adst=np.ascontiguousarray(a_dst[h][:, None]),
        ))
    res = run_bass_kernel_spmd(nc, in_maps, core_ids=list(range(8))).results
    x1 = np.empty((N, H * D), np.float32)
    for c in range(8):
        h, half = c // 2, c % 2
        i0 = half * NH
        x1[i0:i0 + NH, h * D:(h + 1) * D] = res[c]["out"]
    return x1
